# revision 1
# baseline (speedup 1.0000x reference)
"""3-layer GCN encoder (GCNConv+BN+ReLU x3) on 8 Trainium2 NeuronCores.

Strategy (graph/data-parallel over destination nodes):
  - Nodes padded 50000 -> 50176 = 8 * 6272; core c owns dst rows
    [c*6272, (c+1)*6272) = 49 blocks of 128.
  - Per layer l: each core computes its shard of H = X @ W_l channel-major
    on the PE, scales rows by dinv = rsqrt(deg) (table = dinv * (X@W)),
    transposes to node-major, AllGathers the full 50176 x 64 fp32 table.
  - Message passing: edges (incl. self loops) sorted by dst block; per
    block, dma_gather fetches the 256B source rows (int16 indices, so the
    table is addressed in two halves); a one-hot [slot -> dstrel] matmul
    per 128-edge tile segment-sums messages into a [64ch x 128dst] PSUM
    accumulator; the result is scaled by dinv[dst] (GCN symmetric norm,
    with the per-src dinv factored into the table and self loops included
    as ordinary edges).
  - BatchNorm: per-channel sum / sumsq over the local shard via ACT
    accum_out, AllReduce across cores, then one fused
    Relu(S*A + B) activation (A = gamma*rsqrt(var+eps), B = beta - mean*A).
    Conv biases are mathematically absorbed by BN's mean subtraction.
  - Host side does integer index preprocessing only (sort/partition/pad,
    degree counting, layout transposes); all FP math runs on device.
"""
import sys
sys.path.insert(0, "/opt/trn_rl_repo")
import numpy as np

import concourse.bass as bass
import concourse.mybir as mybir
import concourse.tile as tile
from concourse import library_config
from concourse.library_overlay import lower_extended_insts
from concourse.masks import make_identity

N = 50000
NPAD = 50176
NCORES = 8
SHARD = NPAD // NCORES          # 6272
NB = SHARD // 128               # 49 blocks per core
HALF = NPAD // 2                # 25088 (int16-safe table halves)
IN_C = 128
HID = 64
BN_EPS = 1e-5
F32 = mybir.dt.float32
I16 = mybir.dt.int16


def _split_multi_waits(nc, cap=1):
    """walrus in this toolchain accepts one sync wait per instruction;
    hoist extras onto standalone same-engine NOPs."""
    ctr = 0
    for func in nc.m.functions:
        for bb in func.blocks:
            new_insts = []
            for inst in bb.instructions:
                si = inst.sync_info
                if si is not None and len(si.on_wait) > cap:
                    waits = list(si.on_wait)
                    for w in waits[:-cap]:
                        ctr += 1
                        new_insts.append(mybir.InstNoOp(
                            name=f"waitsplit-{ctr}-{inst.name}",
                            sync_info=mybir.SyncInfo(on_wait=[w], on_update=[]),
                            bass_nofuse=True,
                            engine=inst.engine,
                        ))
                    inst.sync_info = mybir.SyncInfo(
                        on_wait=waits[-cap:], on_update=list(si.on_update))
                new_insts.append(inst)
            bb.instructions = new_insts
    return ctr



def _plan(TA, TB):
    """Group blocks in pairs; per group two gather calls (A half, B half),
    each covering the paired blocks' tiles contiguously.
    Returns (groups, SUMT): groups = list of dicts with
      blocks: tuple of block ids
      calls:  [(half, tile_start, ntiles)] two entries
      seg:    {(block, half): (tile_start, ntiles)}
    Tile indices are global (into dstrel / idx column space)."""
    groups = []
    toff = 0
    b = 0
    while b < NB:
        blocks = (b, b + 1) if b + 1 < NB else (b,)
        seg = {}
        callA_start = toff
        for blk in blocks:
            seg[(blk, 0)] = (toff, TA[blk])
            toff += TA[blk]
        callA_n = toff - callA_start
        callB_start = toff
        for blk in blocks:
            seg[(blk, 1)] = (toff, TB[blk])
            toff += TB[blk]
        callB_n = toff - callB_start
        groups.append({
            "blocks": blocks,
            "calls": [(0, callA_start, callA_n), (1, callB_start, callB_n)],
            "seg": seg,
        })
        b += 2
    return groups, toff


def build_kernel(TA, TB, reps=1, do_gather=True, do_compute=True):
    """TA/TB: per-block tile counts (len NB) for the two src halves,
    uniform across cores (max over cores, baked into the program).
    reps>1 replicates the whole 3-layer body (timing instrument only)."""
    groups, SUMT = _plan(TA, TB)

    nc = bass.Bass(num_swdge_queues=4)
    xT_in = nc.dram_tensor("xT", [IN_C, SHARD], F32, kind="ExternalInput")
    degbc_in = nc.dram_tensor("degbc", [HID, SHARD], F32, kind="ExternalInput")
    degnm_in = nc.dram_tensor("degnm", [128, NB], F32, kind="ExternalInput")
    idx_in = nc.dram_tensor("idx", [128, SUMT * 8], I16, kind="ExternalInput")
    dstrel_in = nc.dram_tensor("dstrel", [128, SUMT], F32, kind="ExternalInput")
    iota_in = nc.dram_tensor("iota", [128, 128], F32, kind="ExternalInput")
    w1_in = nc.dram_tensor("w1", [IN_C, HID], F32, kind="ExternalInput")
    w2_in = nc.dram_tensor("w2", [HID, HID], F32, kind="ExternalInput")
    w3_in = nc.dram_tensor("w3", [HID, HID], F32, kind="ExternalInput")
    gb_in = nc.dram_tensor("gb", [HID, 6], F32, kind="ExternalInput")  # g1,be1,g2,be2,g3,be3
    out_t = nc.dram_tensor("outT", [HID, SHARD], F32, kind="ExternalOutput")

    # collective buffers
    ag_in = nc.dram_tensor("ag_in", [SHARD, HID], F32)
    table = nc.dram_tensor("table", [NPAD, HID], F32, addr_space="Shared")
    st_in = nc.dram_tensor("st_in", [HID, 2], F32)
    st_out = nc.dram_tensor("st_out", [HID, 2], F32, addr_space="Shared")

    rgroups = [list(range(NCORES))]

    with tile.TileContext(nc) as tc:
        with (
            tc.tile_pool(name="persist", bufs=1) as pp,
            tc.tile_pool(name="work", bufs=5) as wp,
            tc.tile_pool(name="ohp", bufs=4) as ohpool,
            tc.tile_pool(name="psum", bufs=2, space="PSUM") as psp,
            tc.tile_pool(name="psum_tp", bufs=2, space="PSUM") as ptp,
        ):
            nc.gpsimd.load_library(library_config.mlp)

            # ---- persistent loads ----
            idx = pp.tile([128, SUMT * 8], I16)
            nc.sync.dma_start(idx[:], idx_in[:])
            dstrel = pp.tile([128, SUMT], F32)
            nc.sync.dma_start(dstrel[:], dstrel_in[:])
            iota_t = pp.tile([128, 128], F32)
            nc.sync.dma_start(iota_t[:], iota_in[:])
            w1 = pp.tile([IN_C, HID], F32)
            nc.sync.dma_start(w1[:], w1_in[:])
            w2 = pp.tile([HID, HID], F32)
            nc.sync.dma_start(w2[:], w2_in[:])
            w3 = pp.tile([HID, HID], F32)
            nc.sync.dma_start(w3[:], w3_in[:])
            gb = pp.tile([HID, 6], F32)
            nc.sync.dma_start(gb[:], gb_in[:])
            xT = pp.tile([IN_C, SHARD], F32)
            nc.sync.dma_start(xT[:], xT_in[:])

            ident = pp.tile([HID, HID], F32)
            make_identity(nc, ident[:])
            eps_t = pp.tile([HID, 1], F32)
            nc.vector.memset(eps_t[:], float(BN_EPS))

            # dinv in both layouts: rsqrt(deg) = reciprocal(sqrt(deg))
            dinv_bc = pp.tile([HID, SHARD], F32)
            nc.sync.dma_start(dinv_bc[:], degbc_in[:])
            nc.scalar.sqrt(dinv_bc[:], dinv_bc[:])
            nc.vector.reciprocal(dinv_bc[:], dinv_bc[:])
            dinv_nm = pp.tile([128, NB], F32)
            nc.sync.dma_start(dinv_nm[:], degnm_in[:])
            nc.scalar.sqrt(dinv_nm[:], dinv_nm[:])
            nc.vector.reciprocal(dinv_nm[:], dinv_nm[:])

            # persistent activations (channel-major)
            S_t = pp.tile([HID, SHARD], F32)     # pre-BN conv output
            X_t = pp.tile([HID, SHARD], F32)     # post-BN/ReLU activations
            H_t = pp.tile([HID, SHARD], F32)     # X @ W; reused as stats scratch

            if not do_compute:
                nc.vector.memset(S_t[:], 0.0)

            nreg_cache = {}

            def nreg(v):
                if v not in nreg_cache:
                    nreg_cache[v] = nc.gpsimd.to_reg(v)
                return nreg_cache[v]

            for _rep in range(reps):
              for layer in range(3):
                  w = (w1, w2, w3)[layer]
                  kdim = IN_C if layer == 0 else HID
                  rhs = xT if layer == 0 else X_t

                  # ---- H^T = W^T @ X^T  (channel-major shard matmul) ----
                  col = 0
                  while col < SHARD:
                      nn = min(512, SHARD - col)
                      hp = psp.tile([HID, 512], F32, tag="wmm")
                      nc.tensor.matmul(hp[:, :nn], lhsT=w[:kdim, :], rhs=rhs[:kdim, col:col + nn],
                                       start=True, stop=True)
                      nc.vector.tensor_copy(H_t[:, col:col + nn], hp[:, :nn])
                      col += nn

                  # ---- table shard: transpose to node-major + dinv scale ----
                  for b in range(NB):
                      tp = ptp.tile([128, HID], F32, tag="tp")
                      nc.tensor.transpose(tp[:], H_t[:, b * 128:(b + 1) * 128], ident[:])
                      nm = wp.tile([128, HID], F32, tag="nm")
                      nc.vector.tensor_scalar(out=nm[:], in0=tp[:],
                                              scalar1=dinv_nm[:, b:b + 1], scalar2=None,
                                              op0=mybir.AluOpType.mult)
                      nc.sync.dma_start(ag_in[b * 128:(b + 1) * 128, :], nm[:])

                  nc.gpsimd.collective_compute(
                      "AllGather", mybir.AluOpType.bypass, replica_groups=rgroups,
                      ins=[ag_in[:]], outs=[table[:]],
                  )

                  # ---- message passing, two gather calls per block pair ----
                  qn = 0
                  for grp in groups:
                      gstart = grp["calls"][0][1]
                      gtiles = sum(n for _, _, n in grp["calls"])
                      msg = wp.tile([128, gtiles * HID], F32, tag="msg")
                      if not do_gather:
                          nc.vector.memset(msg[0:1, 0:2], 0.0)
                      if do_gather:
                          for half, tstart, ntile in grp["calls"]:
                              if ntile == 0:
                                  continue
                              tbl_ap = table[0:HALF, :] if half == 0 else table[HALF:NPAD, :]
                              rel = tstart - gstart
                              nc.gpsimd.dma_gather(
                                  out_ap=msg[:, rel * HID:(rel + ntile) * HID]
                                      .rearrange("p (n d) -> p n d", d=HID),
                                  in_ap=tbl_ap,
                                  idxs_ap=idx[:, tstart * 8:(tstart + ntile) * 8],
                                  num_idxs=ntile * 128, num_idxs_reg=nreg(ntile * 128),
                                  elem_size=HID, single_packet=False, queue_num=qn % 4,
                              )
                              qn += 1
                      for blk in grp["blocks"]:
                          if not do_compute:
                              continue
                          ps = psp.tile([HID, 128], F32, tag="scat")
                          segs = [grp["seg"][(blk, 0)], grp["seg"][(blk, 1)]]
                          ntot = sum(n for _, n in segs)
                          ti = 0
                          for tstart, ntile in segs:
                              for t in range(ntile):
                                  gcol = tstart + t
                                  rel = gcol - gstart
                                  oh = ohpool.tile([128, 128], F32, tag="oh")
                                  nc.vector.tensor_scalar(
                                      out=oh[:], in0=iota_t[:],
                                      scalar1=dstrel[:, gcol:gcol + 1], scalar2=None,
                                      op0=mybir.AluOpType.is_equal)
                                  nc.tensor.matmul(ps[:], lhsT=msg[:, rel * HID:(rel + 1) * HID],
                                                   rhs=oh[:],
                                                   start=(ti == 0), stop=(ti == ntot - 1))
                                  ti += 1
                          if do_compute:
                              nc.vector.tensor_tensor(
                                  out=S_t[:, blk * 128:(blk + 1) * 128], in0=ps[:],
                                  in1=dinv_bc[:, blk * 128:(blk + 1) * 128],
                                  op=mybir.AluOpType.mult)

                  # ---- BN stats (local) ----
                  sums = wp.tile([HID, 2], F32, tag="sums")
                  nc.scalar.activation(H_t[:], S_t[:], mybir.ActivationFunctionType.Identity,
                                       accum_out=sums[:, 0:1])
                  nc.scalar.activation(H_t[:], S_t[:], mybir.ActivationFunctionType.Square,
                                       accum_out=sums[:, 1:2])
                  nc.sync.dma_start(st_in[:], sums[:])
                  nc.gpsimd.collective_compute(
                      "AllReduce", mybir.AluOpType.add, replica_groups=rgroups,
                      ins=[st_in[:]], outs=[st_out[:]],
                  )
                  gsums = wp.tile([HID, 2], F32, tag="gsums")
                  nc.sync.dma_start(gsums[:], st_out[:])

                  # mean/var -> A = g*rsqrt(var+eps), B = be - mean*A
                  stat = wp.tile([HID, 4], F32, tag="stat")
                  nc.vector.tensor_scalar(out=stat[:, 0:2], in0=gsums[:], scalar1=1.0 / N,
                                          scalar2=None, op0=mybir.AluOpType.mult)
                  # var = E[x^2] - mean^2
                  nc.vector.tensor_tensor(out=stat[:, 2:3], in0=stat[:, 0:1],
                                          in1=stat[:, 0:1], op=mybir.AluOpType.mult)
                  nc.vector.tensor_tensor(out=stat[:, 2:3], in0=stat[:, 1:2],
                                          in1=stat[:, 2:3], op=mybir.AluOpType.subtract)
                  # sd = sqrt(var + eps); rinv = 1/sd
                  nc.scalar.activation(stat[:, 3:4], stat[:, 2:3],
                                       mybir.ActivationFunctionType.Sqrt, bias=eps_t[:, 0:1])
                  nc.vector.reciprocal(stat[:, 3:4], stat[:, 3:4])
                  ab = wp.tile([HID, 2], F32, tag="ab")
                  nc.vector.tensor_tensor(out=ab[:, 0:1], in0=stat[:, 3:4],
                                          in1=gb[:, 2 * layer:2 * layer + 1],
                                          op=mybir.AluOpType.mult)
                  nc.vector.tensor_tensor(out=ab[:, 1:2], in0=stat[:, 0:1],
                                          in1=ab[:, 0:1], op=mybir.AluOpType.mult)
                  nc.vector.tensor_tensor(out=ab[:, 1:2],
                                          in0=gb[:, 2 * layer + 1:2 * layer + 2],
                                          in1=ab[:, 1:2], op=mybir.AluOpType.subtract)
                  # X = Relu(S*A + B)
                  nc.scalar.activation(X_t[:], S_t[:], mybir.ActivationFunctionType.Relu,
                                       bias=ab[:, 1:2], scale=ab[:, 0:1])

            nc.sync.dma_start(out_t[:], X_t[:])

    _split_multi_waits(nc)
    lower_extended_insts(nc)
    return nc


def _prep(x, edge_index):
    """Host-side integer preprocessing: shard / sort / pad the edge list."""
    src = np.asarray(edge_index[0], dtype=np.int64)
    dst = np.asarray(edge_index[1], dtype=np.int64)
    loops = np.arange(N, dtype=np.int64)
    src = np.concatenate([src, loops])
    dst = np.concatenate([dst, loops])
    deg = np.bincount(dst, minlength=NPAD).astype(np.float32)
    deg[deg == 0] = 1.0

    order = np.argsort(dst, kind="stable")
    src, dst = src[order], dst[order]
    blk = (dst // 128).astype(np.int64)
    # edges grouped per global block; within block split by src half
    counts = {}
    seg = {}
    bstart = np.searchsorted(blk, np.arange(NPAD // 128 + 1))
    for gb in range(NPAD // 128):
        s, e = bstart[gb], bstart[gb + 1]
        bs, bd = src[s:e], dst[s:e]
        a_mask = bs < HALF
        seg[gb] = (bs[a_mask], bd[a_mask], bs[~a_mask], bd[~a_mask])
        counts[gb] = (a_mask.sum(), (~a_mask).sum())

    TA = [0] * NB
    TB = [0] * NB
    for gb in range(NPAD // 128):
        bloc = gb % NB
        ca, cb = counts[gb]
        TA[bloc] = max(TA[bloc], -(-int(ca) // 128))
        TB[bloc] = max(TB[bloc], -(-int(cb) // 128))
    TA = [max(t, 1) for t in TA]
    TB = [max(t, 1) for t in TB]
    SUMT = sum(TA) + sum(TB)

    groups, SUMT = _plan(TA, TB)
    idx_all = np.zeros((NCORES, 128, SUMT * 8), dtype=np.int16)
    dre_all = np.full((NCORES, 128, SUMT), -1.0, dtype=np.float32)
    for c in range(NCORES):
        for grp in groups:
            for blk in grp["blocks"]:
                gb = c * NB + blk
                sa, da, sb, db = seg[gb]
                for half, (ss, dd) in ((0, (sa, da)), (1, (sb, db))):
                    tstart, T = grp["seg"][(blk, half)]
                    half_off = 0 if half == 0 else HALF
                    nslots = T * 128
                    sl_idx = np.zeros(nslots, dtype=np.int16)
                    sl_dre = np.full(nslots, -1.0, dtype=np.float32)
                    k = len(ss)
                    sl_idx[:k] = (ss - half_off).astype(np.int16)
                    sl_dre[:k] = (dd - gb * 128).astype(np.float32)
                    wr = sl_idx.reshape(nslots // 16, 16).T
                    idx_all[c, :, tstart * 8:(tstart + T) * 8] = np.tile(wr, (8, 1))
                    dre_all[c, :, tstart:tstart + T] = sl_dre.reshape(T, 128).T
    return deg, TA, TB, idx_all, dre_all


_CACHE = {}
_REPS = [1]


def build_and_maps(x, edge_index, w1, b1, g1, be1, w2, b2, g2, be2, w3, b3, g3, be3):
    x = np.asarray(x, dtype=np.float32)
    deg, TA, TB, idx_all, dre_all = _prep(x, edge_index)

    key = (tuple(TA), tuple(TB), _REPS[0])
    if key not in _CACHE:
        _CACHE[key] = build_kernel(TA, TB, reps=_REPS[0])
    nc = _CACHE[key]

    xpad = np.zeros((NPAD, IN_C), dtype=np.float32)
    xpad[:N] = x
    iota = np.broadcast_to(np.arange(128, dtype=np.float32), (128, 128)).copy()
    in_maps = []
    for c in range(NCORES):
        sl = slice(c * SHARD, (c + 1) * SHARD)
        deg_c = deg[sl]
        in_maps.append({
            "xT": np.ascontiguousarray(xpad[sl].T),
            "degbc": np.ascontiguousarray(np.broadcast_to(deg_c, (HID, SHARD))),
            "degnm": np.ascontiguousarray(deg_c.reshape(NB, 128).T),
            "idx": idx_all[c],
            "dstrel": dre_all[c],
            "iota": iota,
            "w1": np.asarray(w1, dtype=np.float32),
            "w2": np.asarray(w2, dtype=np.float32),
            "w3": np.asarray(w3, dtype=np.float32),
            "gb": np.stack([np.asarray(a, dtype=np.float32)
                            for a in (g1, be1, g2, be2, g3, be3)], axis=1),
        })

    return nc, in_maps


def kernel(**inputs):
    nc, in_maps = build_and_maps(**inputs)
    from concourse.bass_utils import run_bass_kernel_spmd
    res = run_bass_kernel_spmd(nc, in_maps, list(range(NCORES)))
    out = np.concatenate([res.results[c]["outT"].T for c in range(NCORES)], axis=0)
    return np.ascontiguousarray(out[:N])



# revision 25
# speedup vs baseline: 1.0640x; 1.0640x over previous
"""3-layer GCN encoder (GCNConv+BN+ReLU x3) on 8 Trainium2 NeuronCores.

Strategy (graph/data-parallel over destination nodes):
  - Nodes padded 50000 -> 50176 = 8 * 6272; core c owns dst rows
    [c*6272, (c+1)*6272) = 49 blocks of 128.
  - Per layer l: each core computes its shard of H = X @ W_l channel-major
    on the PE, scales rows by dinv = rsqrt(deg) (table = dinv * (X@W)),
    transposes to node-major, AllGathers the full 50176 x 64 fp32 table.
  - Message passing: edges (incl. self loops) sorted by dst block; per
    block, dma_gather fetches the 256B source rows (int16 indices, so the
    table is addressed in two halves); a one-hot [slot -> dstrel] matmul
    per 128-edge tile segment-sums messages into a [64ch x 128dst] PSUM
    accumulator; the result is scaled by dinv[dst] (GCN symmetric norm,
    with the per-src dinv factored into the table and self loops included
    as ordinary edges).
  - BatchNorm: per-channel sum / sumsq over the local shard via ACT
    accum_out, AllReduce across cores, then one fused
    Relu(S*A + B) activation (A = gamma*rsqrt(var+eps), B = beta - mean*A).
    Conv biases are mathematically absorbed by BN's mean subtraction.
  - Host side does integer index preprocessing only (sort/partition/pad,
    degree counting, layout transposes); all FP math runs on device.
"""
import sys
sys.path.insert(0, "/opt/trn_rl_repo")
import numpy as np

import concourse.bass as bass
import concourse.mybir as mybir
import concourse.tile as tile
from concourse import library_config
from concourse.library_overlay import lower_extended_insts
from concourse.masks import make_identity

N = 50000
NPAD = 50176
NCORES = 8
SHARD = NPAD // NCORES          # 6272
NB = SHARD // 128               # 49 blocks per core
HALF = NPAD // 2                # 25088 (int16-safe table halves)
IN_C = 128
HID = 64
BN_EPS = 1e-5
F32 = mybir.dt.float32
I16 = mybir.dt.int16


def _split_multi_waits(nc, cap=1):
    """walrus in this toolchain accepts one sync wait per instruction;
    hoist extras onto standalone same-engine NOPs."""
    ctr = 0
    for func in nc.m.functions:
        for bb in func.blocks:
            new_insts = []
            for inst in bb.instructions:
                si = inst.sync_info
                if si is not None and len(si.on_wait) > cap:
                    waits = list(si.on_wait)
                    for w in waits[:-cap]:
                        ctr += 1
                        new_insts.append(mybir.InstNoOp(
                            name=f"waitsplit-{ctr}-{inst.name}",
                            sync_info=mybir.SyncInfo(on_wait=[w], on_update=[]),
                            bass_nofuse=True,
                            engine=inst.engine,
                        ))
                    inst.sync_info = mybir.SyncInfo(
                        on_wait=waits[-cap:], on_update=list(si.on_update))
                new_insts.append(inst)
            bb.instructions = new_insts
    return ctr



def _plan(TA, TB, gs=2):
    """Group blocks in chunks of gs; per group two gather calls (A half, B
    half), each covering the grouped blocks' tiles contiguously.
    Returns (groups, SUMT): groups = list of dicts with
      blocks: tuple of block ids
      calls:  [(half, tile_start, ntiles)] two entries
      seg:    {(block, half): (tile_start, ntiles)}
    Tile indices are global (into dstrel / idx column space)."""
    groups = []
    toff = 0
    b = 0
    while b < NB:
        blocks = tuple(range(b, min(b + gs, NB)))
        seg = {}
        callA_start = toff
        for blk in blocks:
            seg[(blk, 0)] = (toff, TA[blk])
            toff += TA[blk]
        callA_n = toff - callA_start
        callB_start = toff
        for blk in blocks:
            seg[(blk, 1)] = (toff, TB[blk])
            toff += TB[blk]
        callB_n = toff - callB_start
        groups.append({
            "blocks": blocks,
            "calls": [(0, callA_start, callA_n), (1, callB_start, callB_n)],
            "seg": seg,
        })
        b += gs
    return groups, toff


def build_kernel(TA, TB, reps=1, do_gather=True, do_compute=True,
                 do_ag=True, do_ar=True, do_mm=True,
                 ag_small=False, single_packet=False, wp_bufs=5,
                 gather_mode="swdge", group_size=2, msg_bufs=None):
    """TA/TB: per-block tile counts (len NB) for the two src halves,
    uniform across cores (max over cores, baked into the program).
    reps>1 replicates the whole 3-layer body (timing instrument only).
    do_* flags disable stages for ablation timing (numerics garbage)."""
    groups, SUMT = _plan(TA, TB, gs=group_size)
    if msg_bufs is None:
        msg_bufs = max(2, 10 // group_size)

    nc = bass.Bass(num_swdge_queues=4)
    xT_in = nc.dram_tensor("xT", [IN_C, SHARD], F32, kind="ExternalInput")
    degbc_in = nc.dram_tensor("degbc", [HID, SHARD], F32, kind="ExternalInput")
    degnm_in = nc.dram_tensor("degnm", [128, NB], F32, kind="ExternalInput")
    if gather_mode == "indirect":
        idx32_in = nc.dram_tensor("idx32", [128, SUMT], mybir.dt.int32,
                                  kind="ExternalInput")
    idx_in = nc.dram_tensor("idx", [128, SUMT * 8], I16, kind="ExternalInput")
    dstrel_in = nc.dram_tensor("dstrel", [128, SUMT], F32, kind="ExternalInput")
    iota_in = nc.dram_tensor("iota", [128, 128], F32, kind="ExternalInput")
    w1_in = nc.dram_tensor("w1", [IN_C, HID], F32, kind="ExternalInput")
    w2_in = nc.dram_tensor("w2", [HID, HID], F32, kind="ExternalInput")
    w3_in = nc.dram_tensor("w3", [HID, HID], F32, kind="ExternalInput")
    gb_in = nc.dram_tensor("gb", [HID, 6], F32, kind="ExternalInput")  # g1,be1,g2,be2,g3,be3
    out_t = nc.dram_tensor("outT", [HID, SHARD], F32, kind="ExternalOutput")

    # collective buffers
    ag_in = nc.dram_tensor("ag_in", [SHARD, HID], F32)
    table = nc.dram_tensor("table", [NPAD, HID], F32, addr_space="Shared")
    st_in = nc.dram_tensor("st_in", [HID, 2], F32)
    st_out = nc.dram_tensor("st_out", [HID, 2], F32, addr_space="Shared")
    ags_in = nc.dram_tensor("ags_in", [HID, 2], F32)
    ags_out = nc.dram_tensor("ags_out", [HID * 8, 2], F32, addr_space="Shared")

    rgroups = [list(range(NCORES))]

    with tile.TileContext(nc) as tc:
        with (
            tc.tile_pool(name="persist", bufs=1) as pp,
            tc.tile_pool(name="work", bufs=wp_bufs) as wp,
            tc.tile_pool(name="ohp", bufs=4) as ohpool,
            tc.tile_pool(name="psum", bufs=2, space="PSUM") as psp,
            tc.tile_pool(name="psum_tp", bufs=2, space="PSUM") as ptp,
        ):
            nc.gpsimd.load_library(library_config.mlp)

            # ---- persistent loads ----
            if gather_mode == "indirect":
                idx32 = pp.tile([128, SUMT], mybir.dt.int32)
                nc.sync.dma_start(idx32[:], idx32_in[:])
            idx = pp.tile([128, SUMT * 8], I16)
            nc.sync.dma_start(idx[:], idx_in[:])
            dstrel = pp.tile([128, SUMT], F32)
            nc.sync.dma_start(dstrel[:], dstrel_in[:])
            iota_t = pp.tile([128, 128], F32)
            nc.sync.dma_start(iota_t[:], iota_in[:])
            w1 = pp.tile([IN_C, HID], F32)
            nc.sync.dma_start(w1[:], w1_in[:])
            w2 = pp.tile([HID, HID], F32)
            nc.sync.dma_start(w2[:], w2_in[:])
            w3 = pp.tile([HID, HID], F32)
            nc.sync.dma_start(w3[:], w3_in[:])
            gb = pp.tile([HID, 6], F32)
            nc.sync.dma_start(gb[:], gb_in[:])
            xT = pp.tile([IN_C, SHARD], F32)
            nc.sync.dma_start(xT[:], xT_in[:])

            ident = pp.tile([HID, HID], F32)
            make_identity(nc, ident[:])
            eps_t = pp.tile([HID, 1], F32)
            nc.vector.memset(eps_t[:], float(BN_EPS))

            # dinv in both layouts: rsqrt(deg) = reciprocal(sqrt(deg))
            dinv_bc = pp.tile([HID, SHARD], F32)
            nc.sync.dma_start(dinv_bc[:], degbc_in[:])
            nc.scalar.sqrt(dinv_bc[:], dinv_bc[:])
            nc.vector.reciprocal(dinv_bc[:], dinv_bc[:])
            dinv_nm = pp.tile([128, NB], F32)
            nc.sync.dma_start(dinv_nm[:], degnm_in[:])
            nc.scalar.sqrt(dinv_nm[:], dinv_nm[:])
            nc.vector.reciprocal(dinv_nm[:], dinv_nm[:])

            # persistent activations (channel-major)
            S_t = pp.tile([HID, SHARD], F32)     # pre-BN conv output
            X_t = pp.tile([HID, SHARD], F32)     # post-BN/ReLU activations
            H_t = pp.tile([HID, SHARD], F32)     # X @ W; reused as stats scratch

            if not do_compute:
                nc.vector.memset(S_t[:], 0.0)
            if reps == 0 or not do_compute:
                nc.vector.memset(X_t[:], 0.0)
            if not do_mm:
                nc.vector.memset(H_t[:], 0.0)

            nreg_cache = {}

            def nreg(v):
                if v not in nreg_cache:
                    nreg_cache[v] = nc.gpsimd.to_reg(v)
                return nreg_cache[v]

            for _rep in range(reps):
              for layer in range(3):
                  w = (w1, w2, w3)[layer]
                  kdim = IN_C if layer == 0 else HID
                  rhs = xT if layer == 0 else X_t

                  # ---- H^T = W^T @ X^T  (channel-major shard matmul) ----
                  col = 0
                  while col < SHARD:
                      nn = min(512, SHARD - col)
                      if do_mm:
                          hp = psp.tile([HID, 512], F32, tag="wmm")
                          nc.tensor.matmul(hp[:, :nn], lhsT=w[:kdim, :], rhs=rhs[:kdim, col:col + nn],
                                           start=True, stop=True)
                          nc.vector.tensor_copy(H_t[:, col:col + nn], hp[:, :nn])
                      col += nn

                  # ---- table shard: transpose to node-major + dinv scale ----
                  for b in range(NB):
                      tp = ptp.tile([128, HID], F32, tag="tp")
                      nc.tensor.transpose(tp[:], H_t[:, b * 128:(b + 1) * 128], ident[:])
                      nm = wp.tile([128, HID], F32, tag="nm")
                      nc.vector.tensor_scalar(out=nm[:], in0=tp[:],
                                              scalar1=dinv_nm[:, b:b + 1], scalar2=None,
                                              op0=mybir.AluOpType.mult)
                      nc.sync.dma_start(ag_in[b * 128:(b + 1) * 128, :], nm[:])

                  if do_ag and ag_small:
                      nc.gpsimd.collective_compute(
                          "AllGather", mybir.AluOpType.bypass, replica_groups=rgroups,
                          ins=[ags_in[:]], outs=[ags_out[:]],
                      )
                  elif do_ag:
                      nc.gpsimd.collective_compute(
                          "AllGather", mybir.AluOpType.bypass, replica_groups=rgroups,
                          ins=[ag_in[:]], outs=[table[:]],
                      )

                  # ---- message passing, two gather calls per block pair ----
                  qn = 0
                  for grp in groups:
                      gstart = grp["calls"][0][1]
                      gtiles = sum(n for _, _, n in grp["calls"])
                      msg = wp.tile([128, gtiles * HID], F32, tag="msg",
                                    bufs=msg_bufs)
                      if not do_gather:
                          nc.vector.memset(msg[0:1, 0:2], 0.0)
                      if do_gather and gather_mode == "indirect":
                          nc.gpsimd.indirect_dma_start(
                              out=msg[:, 0:gtiles * HID]
                                  .rearrange("p (n d) -> p n d", d=HID),
                              out_offset=None,
                              in_=table[:, :],
                              in_offset=bass.IndirectOffsetOnAxis(
                                  ap=idx32[:, gstart:gstart + gtiles], axis=0),
                          )
                      elif do_gather:
                          for half, tstart, ntile in grp["calls"]:
                              if ntile == 0:
                                  continue
                              tbl_ap = table[0:HALF, :] if half == 0 else table[HALF:NPAD, :]
                              rel = tstart - gstart
                              nc.gpsimd.dma_gather(
                                  out_ap=msg[:, rel * HID:(rel + ntile) * HID]
                                      .rearrange("p (n d) -> p n d", d=HID),
                                  in_ap=tbl_ap,
                                  idxs_ap=idx[:, tstart * 8:(tstart + ntile) * 8],
                                  num_idxs=ntile * 128, num_idxs_reg=nreg(ntile * 128),
                                  elem_size=HID, single_packet=single_packet,
                                  queue_num=qn % 4,
                              )
                              qn += 1
                      for blk in grp["blocks"]:
                          if not do_compute:
                              continue
                          ps = psp.tile([HID, 128], F32, tag="scat")
                          segs = [grp["seg"][(blk, 0)], grp["seg"][(blk, 1)]]
                          ntot = sum(n for _, n in segs)
                          ti = 0
                          for tstart, ntile in segs:
                              for t in range(ntile):
                                  gcol = tstart + t
                                  rel = gcol - gstart
                                  oh = ohpool.tile([128, 128], F32, tag="oh")
                                  nc.vector.tensor_scalar(
                                      out=oh[:], in0=iota_t[:],
                                      scalar1=dstrel[:, gcol:gcol + 1], scalar2=None,
                                      op0=mybir.AluOpType.is_equal)
                                  nc.tensor.matmul(ps[:], lhsT=msg[:, rel * HID:(rel + 1) * HID],
                                                   rhs=oh[:],
                                                   start=(ti == 0), stop=(ti == ntot - 1))
                                  ti += 1
                          if do_compute:
                              nc.vector.tensor_tensor(
                                  out=S_t[:, blk * 128:(blk + 1) * 128], in0=ps[:],
                                  in1=dinv_bc[:, blk * 128:(blk + 1) * 128],
                                  op=mybir.AluOpType.mult)

                  # ---- BN stats (local) ----
                  sums = wp.tile([HID, 2], F32, tag="sums")
                  nc.scalar.activation(H_t[:], S_t[:], mybir.ActivationFunctionType.Identity,
                                       accum_out=sums[:, 0:1])
                  nc.scalar.activation(H_t[:], S_t[:], mybir.ActivationFunctionType.Square,
                                       accum_out=sums[:, 1:2])
                  nc.sync.dma_start(st_in[:], sums[:])
                  if do_ar:
                      nc.gpsimd.collective_compute(
                          "AllReduce", mybir.AluOpType.add, replica_groups=rgroups,
                          ins=[st_in[:]], outs=[st_out[:]],
                      )
                  gsums = wp.tile([HID, 2], F32, tag="gsums")
                  nc.sync.dma_start(gsums[:], st_out[:])

                  # mean/var -> A = g*rsqrt(var+eps), B = be - mean*A
                  stat = wp.tile([HID, 4], F32, tag="stat")
                  nc.vector.tensor_scalar(out=stat[:, 0:2], in0=gsums[:], scalar1=1.0 / N,
                                          scalar2=None, op0=mybir.AluOpType.mult)
                  # var = E[x^2] - mean^2
                  nc.vector.tensor_tensor(out=stat[:, 2:3], in0=stat[:, 0:1],
                                          in1=stat[:, 0:1], op=mybir.AluOpType.mult)
                  nc.vector.tensor_tensor(out=stat[:, 2:3], in0=stat[:, 1:2],
                                          in1=stat[:, 2:3], op=mybir.AluOpType.subtract)
                  # sd = sqrt(var + eps); rinv = 1/sd
                  nc.scalar.activation(stat[:, 3:4], stat[:, 2:3],
                                       mybir.ActivationFunctionType.Sqrt, bias=eps_t[:, 0:1])
                  nc.vector.reciprocal(stat[:, 3:4], stat[:, 3:4])
                  ab = wp.tile([HID, 2], F32, tag="ab")
                  nc.vector.tensor_tensor(out=ab[:, 0:1], in0=stat[:, 3:4],
                                          in1=gb[:, 2 * layer:2 * layer + 1],
                                          op=mybir.AluOpType.mult)
                  nc.vector.tensor_tensor(out=ab[:, 1:2], in0=stat[:, 0:1],
                                          in1=ab[:, 0:1], op=mybir.AluOpType.mult)
                  nc.vector.tensor_tensor(out=ab[:, 1:2],
                                          in0=gb[:, 2 * layer + 1:2 * layer + 2],
                                          in1=ab[:, 1:2], op=mybir.AluOpType.subtract)
                  # X = Relu(S*A + B)
                  nc.scalar.activation(X_t[:], S_t[:], mybir.ActivationFunctionType.Relu,
                                       bias=ab[:, 1:2], scale=ab[:, 0:1])

            nc.sync.dma_start(out_t[:], X_t[:])

    _split_multi_waits(nc)
    lower_extended_insts(nc)
    return nc


def _prep(x, edge_index, gs=2):
    """Host-side integer preprocessing: shard / sort / pad the edge list."""
    src = np.asarray(edge_index[0], dtype=np.int64)
    dst = np.asarray(edge_index[1], dtype=np.int64)
    loops = np.arange(N, dtype=np.int64)
    src = np.concatenate([src, loops])
    dst = np.concatenate([dst, loops])
    deg = np.bincount(dst, minlength=NPAD).astype(np.float32)
    deg[deg == 0] = 1.0

    order = np.argsort(dst, kind="stable")
    src, dst = src[order], dst[order]
    blk = (dst // 128).astype(np.int64)
    # edges grouped per global block; within block split by src half
    counts = {}
    seg = {}
    bstart = np.searchsorted(blk, np.arange(NPAD // 128 + 1))
    for gb in range(NPAD // 128):
        s, e = bstart[gb], bstart[gb + 1]
        bs, bd = src[s:e], dst[s:e]
        a_mask = bs < HALF
        seg[gb] = (bs[a_mask], bd[a_mask], bs[~a_mask], bd[~a_mask])
        counts[gb] = (a_mask.sum(), (~a_mask).sum())

    TA = [0] * NB
    TB = [0] * NB
    for gb in range(NPAD // 128):
        bloc = gb % NB
        ca, cb = counts[gb]
        TA[bloc] = max(TA[bloc], -(-int(ca) // 128))
        TB[bloc] = max(TB[bloc], -(-int(cb) // 128))
    TA = [max(t, 1) for t in TA]
    TB = [max(t, 1) for t in TB]
    SUMT = sum(TA) + sum(TB)

    groups, SUMT = _plan(TA, TB, gs=gs)
    idx_all = np.zeros((NCORES, 128, SUMT * 8), dtype=np.int16)
    idx32_all = np.zeros((NCORES, 128, SUMT), dtype=np.int32)
    dre_all = np.full((NCORES, 128, SUMT), -1.0, dtype=np.float32)
    for c in range(NCORES):
        for grp in groups:
            for blk in grp["blocks"]:
                gb = c * NB + blk
                sa, da, sb, db = seg[gb]
                for half, (ss, dd) in ((0, (sa, da)), (1, (sb, db))):
                    tstart, T = grp["seg"][(blk, half)]
                    half_off = 0 if half == 0 else HALF
                    nslots = T * 128
                    sl_idx = np.zeros(nslots, dtype=np.int16)
                    sl_idx32 = np.zeros(nslots, dtype=np.int32)
                    sl_dre = np.full(nslots, -1.0, dtype=np.float32)
                    k = len(ss)
                    sl_idx[:k] = (ss - half_off).astype(np.int16)
                    sl_idx32[:k] = ss.astype(np.int32)
                    sl_dre[:k] = (dd - gb * 128).astype(np.float32)
                    wr = sl_idx.reshape(nslots // 16, 16).T
                    idx_all[c, :, tstart * 8:(tstart + T) * 8] = np.tile(wr, (8, 1))
                    idx32_all[c, :, tstart:tstart + T] = sl_idx32.reshape(T, 128).T
                    dre_all[c, :, tstart:tstart + T] = sl_dre.reshape(T, 128).T
    return deg, TA, TB, idx_all, idx32_all, dre_all


_CACHE = {}
_REPS = [1]
_GATHER_MODE = ["swdge"]
_GS = [2]


def build_and_maps(x, edge_index, w1, b1, g1, be1, w2, b2, g2, be2, w3, b3, g3, be3):
    x = np.asarray(x, dtype=np.float32)
    deg, TA, TB, idx_all, idx32_all, dre_all = _prep(x, edge_index, gs=_GS[0])

    key = (tuple(TA), tuple(TB), _REPS[0], _GATHER_MODE[0], _GS[0])
    if key not in _CACHE:
        _CACHE[key] = build_kernel(TA, TB, reps=_REPS[0],
                                   gather_mode=_GATHER_MODE[0],
                                   group_size=_GS[0])
    nc = _CACHE[key]

    xpad = np.zeros((NPAD, IN_C), dtype=np.float32)
    xpad[:N] = x
    iota = np.broadcast_to(np.arange(128, dtype=np.float32), (128, 128)).copy()
    in_maps = []
    for c in range(NCORES):
        sl = slice(c * SHARD, (c + 1) * SHARD)
        deg_c = deg[sl]
        in_maps.append({
            "xT": np.ascontiguousarray(xpad[sl].T),
            "degbc": np.ascontiguousarray(np.broadcast_to(deg_c, (HID, SHARD))),
            "degnm": np.ascontiguousarray(deg_c.reshape(NB, 128).T),
            "idx": idx_all[c],
            "idx32": idx32_all[c],
            "dstrel": dre_all[c],
            "iota": iota,
            "w1": np.asarray(w1, dtype=np.float32),
            "w2": np.asarray(w2, dtype=np.float32),
            "w3": np.asarray(w3, dtype=np.float32),
            "gb": np.stack([np.asarray(a, dtype=np.float32)
                            for a in (g1, be1, g2, be2, g3, be3)], axis=1),
        })

    return nc, in_maps


def kernel(**inputs):
    nc, in_maps = build_and_maps(**inputs)
    from concourse.bass_utils import run_bass_kernel_spmd
    res = run_bass_kernel_spmd(nc, in_maps, list(range(NCORES)))
    out = np.concatenate([res.results[c]["outT"].T for c in range(NCORES)], axis=0)
    return np.ascontiguousarray(out[:N])

